# revision 2
# baseline (speedup 1.0000x reference)
"""Bilateral filter (2,3,384,384) k=9 on 8 trn2 cores — fp8 DoubleRow version.

Moment expansion (see kernel docstring of the fp16 baseline): three 9x9
Gaussian blurs of [x, x^2, g x^3] + a per-pixel rational combine.

v2 changes vs the fp16 baseline:
  * Mirrored vertical tap pairs (dh, 8-dh) share band weights; one e4m3
    DoubleRow matmul contracts both taps at once (0.5 cyc/col).
  * Powers are precomputed on the host and shipped as fp8 (x^2 scaled x4,
    g x^3 scaled x64 to stay in fp8 normal range; per-slot drain descale).
  * The x-moment drains RAW S1 (no in-matmul center subtraction) so the
    dominant center term uses the fp16 x, not the fp8 one; center-tap
    matmuls run as plain fp8 MMs in e3m4 (finer mantissa, walrus only
    rejects e3m4 for DoubleRow).
  * npow trim T_B: pairs (0,8)->x only, (1,7)->x,x^2 (sim rel err 9.2e-3
    vs 2e-2 budget).
  * Bands quantized with greedy per-row sum correction at scale
    SC = 15/(1-k1n4^2) (center diag == -15.0 exact on both fp8 grids).
  * 12-matmul junk burst with no data deps ramps the PE clock (HAM)
    during the NRT preamble; input DMAs spread over 4 queues.
  * Combine merged over tile pairs {0,1} (free dim 576) with u1/m2 on
    the gpsimd engine in parallel with the DVE chain.
"""

import numpy as np

F16 = np.float16

B, C, H, W = 2, 3, 384, 384
KS = 9
PAD = 4
SIGMA = 0.3 * ((KS - 1) / 2.0 - 1) + 0.8
C2 = 2.0 * SIGMA * SIGMA
NCORES = 8
HPER = H // NCORES                        # 48
NIMG = B * C                              # 6
NT = 4
WIN = 104
WOUT = 96
HPAD = HPER + 2 * PAD                     # 56
SLABF = NIMG * HPAD                       # 336
OUTF = NIMG * HPER                        # 288
HH = HPER // 2                            # 24
M = 112                                   # padded output partitions

C0_FIT = 0.996933770150954
C1_FIT = -0.15881275327745165
GAMMA = C1_FIT / C0_FIT
RA = 2.0977353861724675
RB = -1.0989010989010988

_ax = np.arange(KS, dtype=np.float64) - KS // 2
_k1 = np.exp(-(_ax ** 2) / C2)
K1N = (_k1 / _k1.sum()).astype(np.float64)
CEN = 1.0 - K1N[4] * K1N[4]
SC = 15.0 / CEN                           # center diag -> -15.0 exact
MSCALE = 4.0                              # moment prescale (all slots)

NPOW_PAIR = {0: 1, 1: 2, 2: 3, 3: 3}     # pair (d, 8-d) -> moments fed
JUNK_MMS = 8

# fp8 blob per tile row (1008 B); moments pre-scaled x4 so every PSUM
# slot shares the drain scale 1/(4*SC). The fp16 x rides a separate DMA
# on the slow gpsimd queue (only needed by the combine, much later).
BLOB = 1008
OFF_XE4 = 0         # 336B  e4m3 4x
OFF_X2E4 = 336      # 336B  e4m3 4x^2
OFF_A3E4 = 672      # 336B  e4m3 4 g x^3
XFB = 672           # fp16 x bytes per tile row
BANDB = 1120        # 4 x 224 DR pair bands + 112 c0 + 112 c12

_CACHE = {}


# ---------------- host-side fp8 helpers ----------------
def _quant_e4m3(v):
    v = np.asarray(v, np.float64)
    out = np.zeros_like(v)
    ax = np.abs(v)
    sgn = np.sign(v)
    sub = ax < 2.0 ** -6
    out[sub] = np.round(ax[sub] / 2.0 ** -9) * 2.0 ** -9
    nrm = ~sub
    e = np.floor(np.log2(np.maximum(ax, 1e-300)))
    step = 2.0 ** (e - 3)
    out[nrm] = np.round(ax[nrm] / step[nrm]) * step[nrm]
    return sgn * np.minimum(out, 240.0)


def _quant_e3m4(v):
    v = np.asarray(v, np.float64)
    out = np.zeros_like(v)
    ax = np.abs(v)
    sgn = np.sign(v)
    sub = ax < 2.0 ** -2
    out[sub] = np.round(ax[sub] / 2.0 ** -6) * 2.0 ** -6
    nrm = ~sub
    e = np.floor(np.log2(np.maximum(ax, 1e-300)))
    step = 2.0 ** (e - 4)
    out[nrm] = np.round(ax[nrm] / step[nrm]) * step[nrm]
    return sgn * np.minimum(out, 31.0)


def _enc(v, ebits, mbits, bias):
    """Encode exactly-representable (or RNE) values to TRN fp8 bytes."""
    v = np.asarray(v, np.float64)
    s = (np.signbit(v)).astype(np.uint8) << 7
    a = np.abs(v)
    e = np.floor(np.log2(np.maximum(a, 1e-300))).astype(np.int64)
    minexp = 1 - bias
    e = np.clip(e, minexp, None)
    q = np.round(a / 2.0 ** (e - mbits)).astype(np.int64)   # in [2^m, 2^(m+1)]
    ovf = q >= (1 << (mbits + 1))
    q[ovf] >>= 1
    e[ovf] += 1
    sub = a < 2.0 ** minexp
    qs = np.round(a / 2.0 ** (minexp - mbits)).astype(np.int64)
    byte = np.where(
        sub,
        qs,
        ((e + bias).astype(np.int64) << mbits) | (q - (1 << mbits)),
    )
    byte = np.where(a == 0, 0, byte)
    assert byte.max() < 128
    return (s | byte.astype(np.uint8)).astype(np.uint8)


def enc_e4m3(v):
    return _enc(_quant_e4m3(v), 4, 3, 7)


def enc_e3m4(v):
    return _enc(_quant_e3m4(v), 3, 4, 3)


def _sum_correct(row, scale, quant):
    """Quantize row*scale elementwise, greedily flipping roundings to zero
    the row-sum error. Returns quantized (scaled) float values."""
    w = np.asarray(row, np.float64) * scale
    q = quant(w)
    alt = np.where(q > w, quant(w - (q - w) * 1.999), quant(w + (w - q) * 1.999))
    err = q.sum() - w.sum()
    deltas = alt - q
    order = np.argsort(np.abs(deltas))
    for idx in order:
        d = deltas[idx]
        if d != 0 and abs(err + d) < abs(err):
            q[idx] += d
            err += d
    return q


# ---------------- device kernel ----------------
def _build_nc():
    from contextlib import ExitStack

    import concourse.bass as bass
    import concourse.tile as tile
    from concourse import bacc, mybir

    f32 = mybir.dt.float32
    f16 = mybir.dt.float16
    u8 = mybir.dt.uint8
    e4 = mybir.dt.float8e4
    e3 = mybir.dt.float8e3
    Alu = mybir.AluOpType
    Act = mybir.ActivationFunctionType
    DR = mybir.MatmulPerfMode.DoubleRow

    class DedupBacc(bacc.Bacc):
        """Drop redundant Ldweights when consecutive matmuls share the same
        stationary (the PE array keeps its weights between matmuls)."""

        def move_matmul_waits_to_ldweights(self):
            super().move_matmul_waits_to_ldweights()
            for bb in self.main_func.blocks:
                prev_key = None
                pending = None
                keep = []
                for ins in list(bb.instructions):
                    is_pe = getattr(ins, "engine", None) == self.tensor.engine
                    if isinstance(ins, mybir.InstLdweights):
                        key = str(ins.ins[0])
                        if key == prev_key:
                            pending = ins
                            continue
                        prev_key = key
                    if is_pe and pending is not None:
                        ins.merge_dependencies_from(pending)
                        pending = None
                    keep.append(ins)
                assert pending is None
                bb.instructions[:] = keep

    nc = DedupBacc("TRN2")
    xs_d = nc.dram_tensor("xs", [WIN, NT * BLOB], u8, kind="ExternalInput")
    xf_d = nc.dram_tensor("xf", [WIN, NT * XFB], u8, kind="ExternalInput")
    bd_d = nc.dram_tensor("bands", [WIN, BANDB], u8, kind="ExternalInput")
    y_d = nc.dram_tensor("y", [WIN, NT * OUTF], f16, kind="ExternalOutput")

    with ExitStack() as ctx:
        tc = ctx.enter_context(tile.TileContext(nc))
        singles = ctx.enter_context(tc.tile_pool(name="singles", bufs=1))
        psum = ctx.enter_context(tc.tile_pool(name="psum", bufs=1, space="PSUM"))

        xt = singles.tile([WIN, NT, BLOB], u8)
        xf = singles.tile([WIN, NT, XFB], u8)
        bt = singles.tile([WIN, BANDB], u8)
        y_sb = singles.tile([WIN, NT, NIMG, HPER], f16)
        junk = singles.tile([WIN, OUTF], f16)

        # PE warm-up: junk matmuls gated only on a vector-queue memset
        # (vector is otherwise idle until the combine); they run during
        # the NRT preamble + input DMA so HAM un-throttles early.
        nc.vector.memset(junk[:, :], 0)
        psall = psum.tile([M, 8, 512], f32, name="psall")
        for i in range(JUNK_MMS):
            nc.tensor.matmul(psall[0:WIN, 0, 0:OUTF], junk[:, 0:WIN],
                             junk[:, :], start=(i == 0),
                             stop=(i == JUNK_MMS - 1))

        # input DMAs: bands + early fp8 tiles on the fast sync HWDGE
        # queue (they gate the matmuls); the fp16 x (combine-only, needed
        # ~6us later) rides the slow gpsimd SWDGE in one transfer
        nc.sync.dma_start(out=bt[:, :], in_=bd_d[:, :])
        nc.sync.dma_start(out=xt[:, 0, :], in_=xs_d[:, 0:BLOB])
        nc.sync.dma_start(out=xt[:, 1, :], in_=xs_d[:, BLOB:2 * BLOB])
        nc.scalar.dma_start(out=xt[:, 2, :], in_=xs_d[:, 2 * BLOB:3 * BLOB])
        nc.scalar.dma_start(out=xt[:, 3, :], in_=xs_d[:, 3 * BLOB:4 * BLOB])
        nc.gpsimd.dma_start(
            out=xf[:, :, :].rearrange("p a b -> p (a b)"), in_=xf_d[:, :])
        xfb_ap = xf[:, :, :]

        xb = xt[:, :, :]        # base uint8 AP
        bb = bt[:, :]

        def mov(dt_, byte_off, dims):
            return bass.AP(tensor=xb.tensor, offset=xb.offset + byte_off,
                           ap=[list(xb.ap[0])] + dims).bitcast(dt_)

        def wgt(dt_, byte_off, dims):
            return bass.AP(tensor=bb.tensor, offset=bb.offset + byte_off,
                           ap=[list(bb.ap[0])] + dims).bitcast(dt_)

        def pair_w(d):
            return wgt(e4, d * 224, [[M, 2], [1, M]])

        def c0_w():
            return wgt(e4, 896, [[1, M]])

        def c12_w():
            return wgt(e4, 1008, [[1, M]])

        def bank(t, hh):
            return lambda lo, hi: psall[:, 2 * t + hh, lo:hi]

        # per-tile matmul passes: each tile's 12 MMs share 6 ldweights;
        # its drains + the pair combine overlap the next tile's MMs
        DRAIN_SCALE = 1.0 / (MSCALE * SC)

        # drain target tiles per PAIR group: [WIN, 2, 3 slots, 6 img, 48 h]
        sgs = [singles.tile([WIN, 2, 3, NIMG, HPER], f16, name=f"sg{g}")
               for g in range(2)]

        def emit_tile_mms(t):
            for hh in range(2):
                nc.tensor.matmul(
                    bank(t, hh)(0, 144),
                    c0_w(),
                    mov(e4, t * BLOB + OFF_XE4 + PAD + hh * HH,
                        [[HPAD, NIMG], [1, HH]]),
                    start=True, stop=False)
            for hh in range(2):
                # start=False: c0's start already zeroed the whole 2KB bank
                # (cayman psum_zero_region is always Size2048 — a second
                # start=True here would wipe c0's partial sums)
                nc.tensor.matmul(
                    bank(t, hh)(144, 432),
                    c12_w(),
                    mov(e4, t * BLOB + OFF_X2E4 + PAD + hh * HH,
                        [[SLABF, 2], [HPAD, NIMG], [1, HH]]),
                    start=False, stop=False)
            for d in (3, 2, 1, 0):
                npow = NPOW_PAIR[d]
                for hh in range(2):
                    nc.tensor.matmul(
                        bank(t, hh)(0, npow * 144),
                        pair_w(d),
                        mov(e4, t * BLOB + OFF_XE4 + d + hh * HH,
                            [[KS - 1 - 2 * d, 2], [HPAD, NIMG * npow],
                             [1, HH]]),
                        start=False, stop=(d == 0),
                        perf_mode=DR)

        def emit_tile_drains(t):
            # ONE drain per (t, hh): all 3 slots share the descale (moments
            # pre-scaled x4 on host), and (slot, img) merges into one AP dim
            sg = sgs[t // 2]
            ti = t % 2
            for hh in range(2):
                base = sg[:, ti, :, :, :]
                dst = bass.AP(
                    tensor=base.tensor,
                    offset=base.offset + hh * HH,
                    ap=[list(base.ap[0]), [HPER, 3 * NIMG], [1, HH]])
                nc.scalar.activation(
                    dst, psall[0:WIN, 2 * t + hh, 0:432],
                    Act.Copy, scale=float(DRAIN_SCALE))

        def emit_pair_combine(g):
            # combine over tiles {2g, 2g+1}, free dim 576, all on DVE
            # (gpsimd offload contends for the shared SBUF port - measured
            # 3x DVE slowdown - so everything stays on the vector engine)
            t0 = 2 * g
            FD = 2 * OUTF
            sg = sgs[g]
            sgb = sg[:, :, :, :, :]

            def slot_ap(slot):
                return bass.AP(
                    tensor=sgb.tensor,
                    offset=sgb.offset + slot * OUTF,
                    ap=[list(sgb.ap[0]), [3 * OUTF, 2], [HPER, NIMG],
                        [1, HPER]])

            s1 = slot_ap(0)
            s2 = slot_ap(1)
            a2 = slot_ap(2)
            # byte strides/offset (bitcast to f16 halves them)
            xh = bass.AP(
                tensor=xfb_ap.tensor,
                offset=xfb_ap.offset + t0 * XFB + 2 * PAD,
                ap=[list(xfb_ap.ap[0]), [XFB, 2], [2 * HPAD, NIMG],
                    [1, 2 * HPER]],
            ).bitcast(f16)

            ct = lambda nm: singles.tile([WIN, FD], f16, tag="ct",
                                         bufs=9, name=nm)
            u = ct("u")
            t1 = ct("t1")
            b2 = ct("b2")
            t23 = ct("t23")
            u1 = ct("u1")
            num = ct("num")
            m2 = ct("m2")
            rc = ct("rc")
            qq = ct("qq")

            nc.vector.tensor_tensor(u[:, :], s1, xh, Alu.subtract)
            nc.vector.tensor_tensor(t1[:, :], xh, u[:, :], Alu.mult)
            nc.vector.tensor_tensor(u1[:, :], u[:, :], a2, Alu.add)
            nc.vector.tensor_tensor(b2[:, :], s2, t1[:, :], Alu.subtract)
            nc.vector.tensor_tensor(t23[:, :], xh, b2[:, :], Alu.mult)
            nc.vector.tensor_scalar_mul(num[:, :], t23[:, :],
                                        float(-3.0 * GAMMA))
            nc.vector.tensor_tensor(m2[:, :], b2[:, :], t1[:, :],
                                    Alu.subtract)
            nc.vector.tensor_scalar(rc[:, :], m2[:, :],
                                    float(RB * GAMMA), float(RA + RB),
                                    Alu.mult, Alu.add)
            nc.vector.tensor_tensor(num[:, :], num[:, :], u1[:, :], Alu.add)
            nc.vector.tensor_tensor(qq[:, :], num[:, :], rc[:, :], Alu.mult)

            if g == 0:
                # one add + one DMA for the whole pair
                ysl = y_sb[:, t0:t0 + 2, :, :]
                nc.vector.tensor_tensor(
                    ysl.rearrange("p a b c -> p (a b c)"), xh, qq[:, :],
                    Alu.add)
                nc.sync.dma_start(
                    out=y_d[:, t0 * OUTF:(t0 + 2) * OUTF],
                    in_=ysl.rearrange("p a b c -> p (a b c)"))
            else:
                # tail pair: per-tile add + store so tile 2's DMA overlaps
                # tile 3's add
                for k in range(2):
                    t = t0 + k
                    xh_t = bass.AP(
                        tensor=xfb_ap.tensor,
                        offset=xfb_ap.offset + t * XFB + 2 * PAD,
                        ap=[list(xfb_ap.ap[0]), [2 * HPAD, NIMG],
                            [1, 2 * HPER]],
                    ).bitcast(f16)
                    ysl = y_sb[:, t, :, :]
                    nc.vector.tensor_tensor(
                        ysl.rearrange("p a b -> p (a b)"), xh_t,
                        qq[:, k * OUTF:(k + 1) * OUTF], Alu.add)
                    dq = nc.sync if k == 0 else nc.scalar
                    dq.dma_start(
                        out=y_d[:, t * OUTF:(t + 1) * OUTF],
                        in_=ysl.rearrange("p a b -> p (a b)"))

        for t in range(NT):
            emit_tile_mms(t)
            emit_tile_drains(t)
            if t % 2 == 1:
                emit_pair_combine(t // 2)

    nc.finalize()
    return nc


def get_nc():
    if "nc" not in _CACHE:
        _CACHE["nc"] = _build_nc()
    return _CACHE["nc"]


# ---------------- host shard / unshard ----------------
def _bands_host():
    out = np.zeros((WIN, BANDB), np.uint8)
    # DR pair bands (e4m3): rows dh in 0..3, dup-interleaved [2, 112]
    for d in range(4):
        row = _sum_correct(K1N[d] * K1N, SC, _quant_e4m3)  # 9 scaled vals
        band = np.zeros((WIN, M), np.float64)
        for m in range(PAD, WIN - PAD):
            for k in range(m - PAD, m + PAD + 1):
                band[k, m] = row[k - m + PAD]
        eb = enc_e4m3(band)
        dup = np.zeros((WIN, 2, M), np.uint8)
        dup[:, 0, :] = eb
        dup[:, 1, :] = eb
        out[:, d * 224:(d + 1) * 224] = dup.reshape(WIN, 224)
    # center bands (e4m3); c12 diag = SC*(k1n4^2 - 1) = -15.0 exact
    row0 = _sum_correct(K1N[4] * K1N, SC, _quant_e4m3)
    row12 = _sum_correct(
        np.concatenate([K1N[4] * K1N[:PAD],
                        [K1N[4] * K1N[4] - 1.0],
                        K1N[4] * K1N[PAD + 1:]]), SC, _quant_e4m3)
    for which, row in ((0, row0), (1, row12)):
        band = np.zeros((WIN, M), np.float64)
        for m in range(PAD, WIN - PAD):
            for k in range(m - PAD, m + PAD + 1):
                band[k, m] = row[k - m + PAD]
        out[:, 896 + which * M:896 + (which + 1) * M] = enc_e4m3(band)
    return out


def host_shard(x):
    x = np.asarray(x, np.float32)
    xpad = np.pad(x, ((0, 0), (0, 0), (PAD, PAD), (PAD, PAD)), mode="reflect")
    xpad = xpad.reshape(NIMG, H + 2 * PAD, W + 2 * PAD)
    x64 = xpad.astype(np.float64)
    pw = {
        'xf16': np.asarray(xpad, F16),
        'xe4': enc_e4m3(MSCALE * x64),
        'x2e4': enc_e4m3(MSCALE * x64 * x64),
        'a3e4': enc_e4m3(MSCALE * GAMMA * x64 ** 3),
    }
    bd = _bands_host()
    in_maps = []
    for core in range(NCORES):
        h0 = core * HPER
        xs = np.zeros((WIN, NT, BLOB), np.uint8)
        xfh = np.zeros((WIN, NT, XFB), np.uint8)
        for t in range(NT):
            c0 = 96 * t

            def sl(arr):  # [6, 56, 104] -> [104, 336]
                s = arr[:, h0:h0 + HPAD, c0:c0 + WIN]
                return s.transpose(2, 0, 1).reshape(WIN, SLABF)

            xfh[:, t, :] = sl(pw['xf16']).view(np.uint8)
            xs[:, t, OFF_XE4:OFF_X2E4] = sl(pw['xe4'])
            xs[:, t, OFF_X2E4:OFF_A3E4] = sl(pw['x2e4'])
            xs[:, t, OFF_A3E4:BLOB] = sl(pw['a3e4'])
        in_maps.append({"xs": xs.reshape(WIN, NT * BLOB),
                        "xf": xfh.reshape(WIN, NT * XFB), "bands": bd})
    return in_maps


def host_unshard(ys):
    out = np.empty((B, C, H, W), np.float32)
    oi = out.reshape(NIMG, H, W)
    for core in range(NCORES):
        h0 = core * HPER
        y = np.asarray(ys[core], np.float32).reshape(WIN, NT, NIMG, HPER)
        y = y[PAD:PAD + WOUT]                      # [96, t, img, h]
        yt = y.transpose(2, 3, 1, 0).reshape(NIMG, HPER, W)
        oi[:, h0:h0 + HPER, :] = yt
    return out


def kernel(x, ksize):
    from concourse.bass_utils import run_bass_kernel_spmd

    assert int(ksize) == KS
    x = np.asarray(x, dtype=np.float32)
    assert x.shape == (B, C, H, W)
    in_maps = host_shard(x)
    nc = get_nc()
    res = run_bass_kernel_spmd(nc, in_maps, core_ids=list(range(NCORES)))
    ys = [np.asarray(r["y"]) for r in res.results]
    return host_unshard(ys)
